# revision 6
# baseline (speedup 1.0000x reference)
"""CenterLoss Trainium2 kernel (fp8 DoubleRow, on-device squaring).

Full inputs:
  ep_mask_embed    (8, 4096, 256) f32
  ep_mask          (8, 1, 1024, 1024) f32
  query_mask_embed (8, 4096, 256) f32
  query_mask       (8, 1, 1024, 1024) f32
Output: (3,) f32 = [mean(center_loss), mean(pos_loss), mean(neg_loss)]

Sharding: data-parallel, one batch sample per NeuronCore (8 cores).

Per sample the loss reduces to epw = [m;1-m]^T ep, qw = [m;1-m]^T q,
qsqw = [m;1-m]^T q^2 plus mask counts; everything downstream is ~50
scalar flops done on host from those statistics (where the batch mean
already happens).

Trace-driven structure (v4):
  - HBM traffic 2MB/core (not 3MB): q^2 is squared on-device instead
    of host-precomputed+streamed.  DVE and ACT (both idle otherwise)
    split each quarter by columns, balanced by their errata-adjusted
    rates ((151+x)/0.96 vs (224+2048-x)/1.2).
  - Tile tracks data deps at WHOLE-TILE granularity, so every unit
    that must release consumers independently is its own tile: q
    chunk 0 arrives as two 256KB tiles (squaring starts ~1.3us
    earlier), each squared quarter is its own tile with exactly its
    two writers (DVE cols [0:926), ACT rest).
  - All stream DMAs ride the SYNC HWDGE ring in consumption order
    (qA, qB, q1, ep0, ep1); the ACT ring stays free so the ~2.7us
    Square ACT_TABLE_LOAD hides under the DMA ramp.
  - PE warm-up: zero matmuls during the DMA ramp flip the HAM clock
    gate (4/8 -> 8/8) so real matmuls run ~109ns instead of 213ns.
  - fp8 re-rounding of q^2 is bias-corrected by per-partition dither:
    squares of the 8 fp8 mantissa patterns land systematically off
    RNE midpoints (-0.5% summed), so partition p squares are scaled
    by s_p (8 values in [0.8,1.23] whose reciprocals are EXACT fp8
    numbers) and the qsq matmul weights carry w/s_p in separate
    weight planes -- the unmixing is exact, the residual rounding
    bias averages out across the dithered partitions.
  - Single out-DMA for all three stat sections.
"""

import numpy as np
import ml_dtypes
from contextlib import ExitStack

import concourse.bass as bass
import concourse.bacc as bacc
import concourse.tile as tile
from concourse import mybir
from concourse.bass_utils import run_bass_kernel_spmd

F32 = mybir.dt.float32
F8 = mybir.dt.float8e4
NP_F8 = ml_dtypes.float8_e4m3fn

P = 128          # partitions
N_TOK = 4096     # tokens per sample (64*64 patches)
C = 256          # channels
T = 16           # tokens per partition per chunk (4KB fp8 descriptor)
DC = P * T       # tokens per chunk (2048)
N_DC = N_TOK // DC   # 2 chunks
NPC = T // 2     # parity-pairs (pieces) per chunk: 8
B = 8            # batch == n cores
PATCH = 16
QCOL = T * C // 2    # columns per quarter tile (2048)
SQ_SPLIT = 926       # within each quarter: DVE squares [0:926), ACT rest
NM_PLANES = 6        # weight planes: q_pos,q_neg,ep_pos,ep_neg,qsq_pos,qsq_neg
LW_COLS = 2 * 128    # two ks planes of 128 cols (6*16 used per plane)
N_WARM = 10          # PE warm-up matmuls (~2.1us of the DMA ramp)

# per-partition dither: reciprocals are exact fp8e4m3 grid points
_INV_S = np.array([0.8125, 0.875, 0.9375, 1.0, 1.0625, 1.125, 1.1875, 1.25])
_S_P = 1.0 / _INV_S[np.arange(P) % 8]          # scale fed to DVE
_SQRT_S_P = np.sqrt(_S_P)                       # scale fed to ACT Square

_CACHE = {}


def _build():
    """Build the per-core Bass program (identical on all cores)."""
    nc = bacc.Bacc("TRN2", target_bir_lowering=False, debug=False)

    ep8 = nc.dram_tensor("ep8", [N_TOK, C], F8, kind="ExternalInput").ap()
    q8 = nc.dram_tensor("q8", [N_TOK, C], F8, kind="ExternalInput").ap()
    # host-packed DoubleRow mask weights.  The dual-fp8 ldweights ISA
    # check needs the dual-row AP dim to have num_elem==2 and a step
    # that is a multiple of 16 elements, so the two ks sub-rows live in
    # separate 128-col planes: col = 128*ks + 6*jj + m,
    # m in (q_pos, q_neg, ep_pos, ep_neg, qsq_pos, qsq_neg),
    # token = 2048*(jj//8) + 16*p + 2*(jj%8) + ks
    lw = nc.dram_tensor("lw", [P, LW_COLS], F8, kind="ExternalInput").ap()
    # [s_p, sqrt(s_p)] dither scales
    sqs = nc.dram_tensor("sqs", [P, 2], F32, kind="ExternalInput").ap()
    # [epw | qw | qsqw], rows = (pos, neg)
    out = nc.dram_tensor("out", [2, 3 * C], F32, kind="ExternalOutput").ap()

    DR = mybir.MatmulPerfMode.DoubleRow

    with tile.TileContext(nc) as tc, ExitStack() as ctx:
        const_pool = ctx.enter_context(tc.tile_pool(name="const", bufs=1))
        x_pool = ctx.enter_context(tc.tile_pool(name="x_pool", bufs=1))
        sq_pool = ctx.enter_context(tc.tile_pool(name="sq_pool", bufs=1))
        psum_pool = ctx.enter_context(
            tc.tile_pool(name="psum", bufs=1, space=bass.MemorySpace.PSUM)
        )
        fin_pool = ctx.enter_context(tc.tile_pool(name="fin", bufs=1))

        lw_t = const_pool.tile([P, LW_COLS], F8, name="lw_t", tag="lw_t")
        nc.sync.dma_start(out=lw_t[:], in_=lw[:])
        sqs_t = const_pool.tile([P, 2], F32, name="sqs_t", tag="sqs_t")
        nc.sync.dma_start(out=sqs_t[:], in_=sqs[:])

        # PE warm-up: zeros tile -> N=256 normal-mode matmuls into a
        # scratch PSUM bank.  No stream dependency, so they run during
        # the DMA ramp and flip HAM to 8/8 before real work arrives.
        warm = const_pool.tile([P, C], F8, name="warm", tag="warm")
        nc.gpsimd.memset(warm[:], 0)
        warm_ps = psum_pool.tile([P, C], F32, name="warm_ps", tag="warm_ps")
        for _ in range(N_WARM):
            nc.tensor.matmul(
                warm_ps[:], warm[:, 0:P], warm[:], start=True, stop=True
            )

        # Stream units in consumption order on the sync ring; each is
        # its OWN tile so its completion releases consumers
        # independently.  (qA, qB) = 256KB halves of q chunk 0.
        def qsrc(i, h0, h1):
            return q8[i * DC:(i + 1) * DC, :].rearrange(
                "(p t) c -> p t c", t=T)[:, h0:h1, :]

        units = []   # (name, n_pieces)
        for name, src, npieces in (
            ("qA", qsrc(0, 0, T // 2), 4),
            ("qB", qsrc(0, T // 2, T), 4),
            ("q1", qsrc(1, 0, T), 8),
            ("ep0", ep8[0:DC, :].rearrange("(p t) c -> p (t c)", t=T), 8),
            ("ep1", ep8[DC:2 * DC, :].rearrange("(p t) c -> p (t c)", t=T), 8),
        ):
            t_ = x_pool.tile([P, npieces * 512], F8, name=name, tag=name)
            nc.sync.dma_start(out=t_[:], in_=src)
            units.append((name, t_))
        U = dict(units)

        # On-device s_p*q^2 (fp8 in/out, fp32 internal), one DVE+ACT op
        # pair per quarter tile as its data lands.  q1 is one 512KB
        # tile; its two quarters still square independently.
        s_ap = sqs_t[:, 0:1]
        rs_ap = sqs_t[:, 1:2]
        SQT = {}
        for qname, base in (("qA", 0), ("qB", 0), ("q1", 0), ("q1", QCOL)):
            key = len(SQT)  # 0..3 = quarters A,B,C,D
            sq = sq_pool.tile([P, QCOL], F8, name=f"sq{key}", tag=f"sq{key}")
            qt = U[qname]
            nc.vector.scalar_tensor_tensor(
                sq[:, 0:SQ_SPLIT],
                qt[:, base:base + SQ_SPLIT],
                s_ap,
                qt[:, base:base + SQ_SPLIT],
                mybir.AluOpType.mult,
                mybir.AluOpType.mult,
            )
            nc.scalar.activation(
                sq[:, SQ_SPLIT:QCOL],
                qt[:, base + SQ_SPLIT:base + QCOL],
                mybir.ActivationFunctionType.Square,
                scale=rs_ap,
            )
            SQT[key] = sq

        psum = {
            nm: psum_pool.tile([2, C], F32, name=f"ps_{nm}", tag=f"ps_{nm}")
            for nm in ("ep", "q", "qsq")
        }

        fin = fin_pool.tile([2, 3 * C], F32, name="fin", tag="fin")
        SEC = {"ep": 0, "q": 1, "qsq": 2}
        WOFF = {"q": 0, "ep": 2, "qsq": 4}

        # PE bursts in expected data-availability order.  Each entry:
        # (chain, source tile, chunk, global piece range).
        bursts = [
            ("q", U["qA"], 0, 0, 4), ("q", U["qB"], 0, 4, 4),
            ("qsq", SQT[0], 0, 0, 4),
            ("qsq", SQT[1], 0, 4, 4),
            ("q", U["q1"], 1, 0, 8),
            ("qsq", SQT[2], 1, 0, 4),
            ("ep", U["ep0"], 0, 0, 8),
            ("qsq", SQT[3], 1, 4, 4),
            ("ep", U["ep1"], 1, 0, 8),
        ]
        for nm, src_t, i, j0, nj in bursts:
            for j in range(j0, j0 + nj):
                jj = NPC * i + j
                off = NM_PLANES * jj + WOFF[nm]
                w = lw_t[:].rearrange(
                    "p (k c) -> p k c", k=2)[:, :, off:off + 2]
                lo = 512 * (j - j0)
                rhs = src_t[:, lo:lo + 512].rearrange(
                    "p (k c) -> p k c", k=2)
                nc.tensor.matmul(
                    psum[nm][:], w, rhs,
                    start=(i == 0 and j == 0),
                    stop=(i == N_DC - 1 and j == NPC - 1),
                    perf_mode=DR,
                )
            if i == N_DC - 1 and j0 + nj == NPC:
                s = SEC[nm]
                # last chain (ep) ships via the by-then-idle DVE; the
                # earlier two via ACT (its squares are done by then).
                fsec = fin[:, s * C:(s + 1) * C]
                if nm == "ep":
                    nc.vector.tensor_copy(fsec, psum[nm][:])
                else:
                    nc.scalar.copy(fsec, psum[nm][:])

        # single out-DMA for all three sections
        nc.sync.dma_start(out=out[:], in_=fin[:])

    nc.compile()
    return nc


def get_nc():
    if "nc" not in _CACHE:
        _CACHE["nc"] = _build()
    return _CACHE["nc"]


# token index per (partition, piece jj, ks): DoubleRow weight layout
_PG = np.arange(P)[:, None, None]
_JJ = np.arange(N_DC * NPC)[None, :, None]
_KS = np.arange(2)[None, None, :]
_TOK = (DC * (_JJ // NPC) + T * _PG + 2 * (_JJ % NPC) + _KS)  # [128, 16, 2]


def _mask_ds(mask_b):
    """Downsample one sample's mask (nearest, stride 16) -> (4096,) f64."""
    return mask_b[0, ::PATCH, ::PATCH].reshape(-1).astype(np.float64)


def make_in_maps(ep_mask_embed, ep_mask, query_mask_embed, query_mask):
    inv_s = _INV_S[np.arange(P) % 8][:, None, None]  # [p, 1, 1]
    sqs = np.stack([_S_P, _SQRT_S_P], axis=1).astype(np.float32)  # [P, 2]
    in_maps, counts = [], []
    for b in range(B):
        em = _mask_ds(ep_mask[b])
        qm = _mask_ds(query_mask[b])
        et = em[_TOK]  # [128, 16, 2] = (p, jj, ks)
        qt = qm[_TOK]
        # weight planes: q_pos, q_neg, ep_pos, ep_neg, qsq_pos/s, qsq_neg/s
        L = np.stack([qt, 1.0 - qt, et, 1.0 - et,
                      qt * inv_s, (1.0 - qt) * inv_s], axis=-1)  # [p,jj,ks,m]
        lw_b = np.zeros((P, 2, 128), dtype=np.float64)
        # col = 128*ks + 6*jj + m
        lw_b[:, :, :NM_PLANES * N_DC * NPC] = (
            L.transpose(0, 2, 1, 3).reshape(P, 2, NM_PLANES * N_DC * NPC))
        in_maps.append({
            "ep8": np.ascontiguousarray(ep_mask_embed[b]).astype(NP_F8),
            "q8": np.ascontiguousarray(query_mask_embed[b]).astype(NP_F8),
            "lw": lw_b.reshape(P, LW_COLS).astype(NP_F8),
            "sqs": sqs,
        })
        counts.append((em.sum(), (1.0 - em).sum(), qm.sum(), (1.0 - qm).sum()))
    return in_maps, counts


def finalize(per_core, counts):
    """per_core: list of 8 arrays [2, 768] (epw|qw|qsqw) -> full (3,)."""
    pos = np.zeros(B)
    neg = np.zeros(B)
    for b in range(B):
        st = np.asarray(per_core[b]).astype(np.float64)
        n_pe, n_ne, n_pq, n_nq = counts[b]
        epw, qw, qsq = st[:, 0:C], st[:, C:2 * C], st[:, 2 * C:3 * C]
        pc = epw[0] / (n_pe + 0.1)
        ncen = epw[1] / (n_ne + 0.1)
        pn = qsq[0].sum() - 2.0 * (pc @ qw[0]) + n_pq * (pc @ pc)
        nn = qsq[1].sum() - 2.0 * (ncen @ qw[1]) + n_nq * (ncen @ ncen)
        pos[b] = pn / (max(n_pq, 1.0) * C) if n_pq > 0 else 0.0
        neg[b] = nn / (max(n_nq, 1.0) * C) if n_nq > 0 else 0.0
    return np.array(
        [(pos + neg).mean(), pos.mean(), neg.mean()], dtype=np.float32
    )


def kernel(ep_mask_embed, ep_mask, query_mask_embed, query_mask):
    ep_mask_embed = np.asarray(ep_mask_embed, dtype=np.float32)
    ep_mask = np.asarray(ep_mask, dtype=np.float32)
    query_mask_embed = np.asarray(query_mask_embed, dtype=np.float32)
    query_mask = np.asarray(query_mask, dtype=np.float32)

    nc = get_nc()
    in_maps, counts = make_in_maps(
        ep_mask_embed, ep_mask, query_mask_embed, query_mask)
    # First execution after device bring-up has been observed to return
    # garbage once; retry on non-finite results.
    for _ in range(3):
        res = run_bass_kernel_spmd(nc, in_maps, list(range(B)))
        result = finalize([r["out"] for r in res.results], counts)
        if np.all(np.isfinite(result)):
            return result
    return result
